# revision 10
# baseline (speedup 1.0000x reference)
"""Multi-head attention (B=2,S=2048,D=1024,H=16,A=64) on 8 trn2 NeuronCores.

Sharding: core = 4*b + g  (b = batch, g = head-group of 4 heads).
Per core: project q/k transposed (feature-on-partition), v natural; per
head-pair S^T tiles -> exp on ACT -> AV accumulate with a ones-column in v
producing the softmax denominator; normalize via fast-reciprocal +
partition-broadcast multiply; fc_out partial (this core's 256 channels of
Wo) interleaved per q-chunk; host sums the 4 partials per batch.

Schedule: attention is ACT(exp)-paced; projections for pair 1, the V
projection, and fc_out chunks are emitted as PE filler inside the
attention chunk loops so the PE never idles waiting on the ACT engine.

ATT_DT: "f32r" = bf16 proj + tf32-ish attention matmuls (safe); "fp8" =
scores matmul in fp8e4 DoubleRow (two 32-row halves packed; Wq/Wk columns
permuted host-side into the [32, 2] layout), rest f32r.
"""

import numpy as np

B, S, D, H, A = 2, 2048, 1024, 16, 64
GROUPS = 4              # head groups (cores per batch)
HPG = H // GROUPS       # heads per core = 4
C = HPG * A             # channels per core = 256
N_CORES = 8
KD = D // 128           # d-tiles = 8
MC = C // 128           # channel tiles per core = 2
NS = S // 128           # seq tiles = 16
QC = 4                  # q chunks
QW = S // QC            # 512
NG = NS // 2            # S^T groups per chunk (2 k-tiles per group)

ATT_DT = "f32r"         # "f32r" or "fp8" (scores matmul dtype)
NORM_MODE = "gp"        # "gp": fast recip + gpsimd broadcast;
                        # "pe": fast recip + PE ones-matmul broadcast;
                        # "safe": exact recip + PE broadcast


def build_nc(att_dt=None):
    import concourse.mybir as mybir
    import concourse.tile as tile
    from concourse import bacc

    att_dt = att_dt or ATT_DT
    fp8 = att_dt == "fp8"
    f32 = mybir.dt.float32
    f32r = mybir.dt.float32r
    bf16 = mybir.dt.bfloat16
    f8 = mybir.dt.float8e4
    AF = mybir.ActivationFunctionType
    DR = mybir.MatmulPerfMode.DoubleRow

    def r(ap):
        return ap

    # proj inputs: f32r in fp8 mode, bf16 (host-cast) otherwise
    pdt = f32r if fp8 else bf16

    def pr(ap):
        return ap

    nc = bacc.Bacc(
        "TRN2", target_bir_lowering=False, debug=False,
        enable_asserts=True, num_devices=N_CORES,
    )

    xT_d = nc.dram_tensor("xT", [D, S], pdt, kind="ExternalInput").ap()
    wq_d = nc.dram_tensor("wq", [D, C], pdt, kind="ExternalInput").ap()
    wk_d = nc.dram_tensor("wk", [D, C], pdt, kind="ExternalInput").ap()
    wv_d = nc.dram_tensor("wv", [D, C], pdt, kind="ExternalInput").ap()
    wo_d = nc.dram_tensor("wo", [C, D], f32r, kind="ExternalInput").ap()
    bqs_d = nc.dram_tensor("bqs", [128, MC], f32, kind="ExternalInput").ap()
    bks_d = nc.dram_tensor("bks", [128, MC], f32, kind="ExternalInput").ap()
    bvb_d = nc.dram_tensor("bvb", [128, C], f32, kind="ExternalInput").ap()
    bob_d = nc.dram_tensor("bob", [128, D], f32, kind="ExternalInput").ap()
    out_d = nc.dram_tensor("out", [S, D], f32, kind="ExternalOutput").ap()

    with tile.TileContext(nc) as tc:
        with tc.tile_pool(name="const", bufs=1) as cpool, \
             tc.tile_pool(name="wgt", bufs=1) as wpool, \
             tc.tile_pool(name="qkv", bufs=1) as qpool, \
             tc.tile_pool(name="ptp", bufs=2) as ptpool, \
             tc.tile_pool(name="rcp", bufs=2) as rpool, \
             tc.tile_pool(name="osb", bufs=1) as opool, \
             tc.tile_pool(name="pst", bufs=1, space="PSUM") as stp, \
             tc.tile_pool(name="pav", bufs=1, space="PSUM") as avp, \
             tc.tile_pool(name="psp", bufs=2, space="PSUM") as psp:

            # ---------------- constants + input loads ----------------
            bq_sb = cpool.tile([128, MC], f32, name="bq_sb")
            nc.sync.dma_start(bq_sb[:], bqs_d[:, :])
            bk_sb = cpool.tile([128, MC], f32, name="bk_sb")
            nc.sync.dma_start(bk_sb[:], bks_d[:, :])
            bvb_sb = cpool.tile([128, C], f32, name="bvb_sb")
            nc.sync.dma_start(bvb_sb[:], bvb_d[:, :])
            bob_sb = cpool.tile([128, D], f32, name="bob_sb")
            nc.sync.dma_start(bob_sb[:], bob_d[:, :])

            xT_sb = [wpool.tile([128, S], pdt, name=f"xT{kt}")
                     for kt in range(KD)]
            w_sb = {w: [wpool.tile([128, C], pdt, name=f"w{w}{kt}")
                        for kt in range(KD)]
                    for w in ("q", "k", "v")}
            wds = {"q": wq_d, "k": wk_d, "v": wv_d}
            for kt in range(KD):
                ks = slice(kt * 128, (kt + 1) * 128)
                nc.sync.dma_start(xT_sb[kt][:], xT_d[ks, :])
                for i, w in enumerate(("q", "k", "v")):
                    eng = (nc.gpsimd, nc.scalar, nc.gpsimd)[i]
                    eng.dma_start(w_sb[w][kt][:], wds[w][ks, :])
            wo_sb = [wpool.tile([128, D], f32r, name=f"wo{kt}")
                     for kt in range(MC)]
            for kt in range(MC):
                nc.sync.dma_start(wo_sb[kt][:],
                                  wo_d[kt * 128:(kt + 1) * 128, :])

            if NORM_MODE != "gp":
                ones_f = cpool.tile([1, A], f32, name="ones_f")
                nc.vector.memset(ones_f[:], 1.0)
                ones_r = cpool.tile([1, A], f32r, name="ones_r")
                nc.vector.tensor_copy(ones_r[:], ones_f[:])

            # v padded per head with a ones column: [128, NS, HPG, A+1]
            v_sb = qpool.tile([128, NS, HPG, A + 1], f32r, name="v_sb")
            vones = cpool.tile([128, NS * HPG], f32, name="vones")
            nc.vector.memset(vones[:], 1.0)
            nc.vector.tensor_copy(
                v_sb[:, :, :, A],
                vones[:].rearrange("p (t h) -> p t h", h=HPG))

            if fp8:
                # all 4 heads in one tile: partition = h*32 + a%32,
                # free = (a//32, s)
                qT_f8 = qpool.tile([128, 2, S], f8, name="qT_f8")
                kT_f8 = qpool.tile([128, 2, S], f8, name="kT_f8")
            else:
                qT_sb = [qpool.tile([128, S], f32r, name=f"qT{p}")
                         for p in range(MC)]
                kT_sb = [qpool.tile([128, S], f32r, name=f"kT{p}")
                         for p in range(MC)]
            attn_sb = [qpool.tile([128, S], f32r, name=f"attn{p}")
                       for p in range(MC)]

            # ---------------- work units ----------------
            def qk_unit(mt, wname, qc):
                # one q-chunk of the q or k projection for tile/half mt
                qs = slice(qc * QW, (qc + 1) * QW)
                ps = psp.tile([128, QW], f32, name="ps", tag="ps")
                for kt in range(KD):
                    nc.tensor.matmul(
                        ps[:],
                        lhsT=pr(w_sb[wname][kt][:, mt * 128:(mt + 1) * 128]),
                        rhs=pr(xT_sb[kt][:, qs]),
                        start=(kt == 0), stop=(kt == KD - 1),
                    )
                bias = (bq_sb if wname == "q" else bk_sb)[:, mt:mt + 1]
                if fp8:
                    dst = qT_f8 if wname == "q" else kT_f8
                    nc.vector.tensor_scalar_add(dst[:, mt, qs], ps[:], bias)
                else:
                    dst = (qT_sb if wname == "q" else kT_sb)[mt]
                    nc.vector.tensor_scalar_add(dst[:, qs], ps[:], bias)

            def v_unit(st):
                # one seq-tile of the v projection (natural layout)
                psv = psp.tile([128, C], f32, name="psv", tag="ps")
                for kt in range(KD):
                    nc.tensor.matmul(
                        psv[:],
                        lhsT=pr(xT_sb[kt][:, st * 128:(st + 1) * 128]),
                        rhs=pr(w_sb["v"][kt][:]),
                        start=(kt == 0), stop=(kt == KD - 1),
                    )
                nc.vector.tensor_add(
                    v_sb[:, st, :, 0:A],
                    psv[:].rearrange("p (h a) -> p h a", a=A),
                    bvb_sb[:].rearrange("p (h a) -> p h a", a=A),
                )

            def fc_unit(mt, nn):
                # fc_out: s-tile mt (128 rows), d-half nn; contraction over
                # this core's C=256 attn channels
                ps = psp.tile([128, QW], f32, name="psfc", tag="ps")
                for kt in range(MC):
                    nc.tensor.matmul(
                        ps[:],
                        lhsT=r(attn_sb[kt][:, mt * 128:(mt + 1) * 128]),
                        rhs=r(wo_sb[kt][:, nn * QW:(nn + 1) * QW]),
                        start=(kt == 0), stop=(kt == MC - 1),
                    )
                ob = ob_tiles[mt % 4]
                nc.vector.tensor_add(ob[:, nn * QW:(nn + 1) * QW], ps[:],
                                     bob_sb[:, nn * QW:(nn + 1) * QW])
                if nn == 1:
                    nc.sync.dma_start(out_d[mt * 128:(mt + 1) * 128, :],
                                      ob[:])

            # filler queue: zero-arg closures popped between ng groups
            # inside attention chunks to keep the PE busy
            filler = []

            def drain(n):
                for _ in range(n):
                    if filler:
                        filler.pop(0)()

            def attention_chunk(p, qc, v_inline=False):
                heads = (2 * p, 2 * p + 1)
                qs = slice(qc * QW, (qc + 1) * QW)
                avs = [avp.tile([A + 1, QW], f32, name=f"av{hh}",
                                tag=f"av{hh}") for hh in range(2)]
                for ng in range(NG):
                    pts = []
                    for hh in range(2):
                        st = stp.tile([128, 2, QW], f32, name=f"st{hh}",
                                      tag=f"st{hh}")
                        for jj in range(2):
                            kt = 2 * ng + jj
                            if fp8:
                                h = heads[hh]
                                nc.tensor.matmul(
                                    st[:, jj, :],
                                    lhsT=kT_f8[h * 32:(h + 1) * 32, :,
                                               kt * 128:(kt + 1) * 128],
                                    rhs=qT_f8[h * 32:(h + 1) * 32, :, qs],
                                    start=True, stop=True, perf_mode=DR,
                                )
                            else:
                                off = hh * A
                                nc.tensor.matmul(
                                    st[:, jj, :],
                                    lhsT=r(kT_sb[p][off:off + A,
                                                    kt * 128:(kt + 1) * 128]),
                                    rhs=r(qT_sb[p][off:off + A, qs]),
                                    start=True, stop=True,
                                )
                        pt = ptpool.tile([128, 2, QW], f32r, name=f"pt{hh}",
                                         tag=f"pt{hh}")
                        nc.scalar.activation(pt[:], st[:], AF.Exp,
                                             scale=0.125)
                        pts.append(pt)
                    if v_inline:
                        v_unit(2 * ng)
                        v_unit(2 * ng + 1)
                    else:
                        drain(1)
                    for jj in range(2):
                        kt = 2 * ng + jj
                        for hh in range(2):
                            nc.tensor.matmul(
                                avs[hh][:],
                                lhsT=r(v_sb[:, kt, heads[hh], :]),
                                rhs=r(pts[hh][:, jj, :]),
                                start=(kt == 0), stop=(kt == NS - 1),
                            )
                # normalize: attn[a, q] = av[a, q] * (1 / av[A, q])
                for hh in range(2):
                    av = avs[hh]
                    off = hh * A
                    rec = rpool.tile([1, QW], f32, name="rec", tag="rec")
                    if NORM_MODE == "safe":
                        nc.vector.reciprocal(rec[:], av[A:A + 1, :])
                    else:
                        # custom-DVE ops drop the input partition offset;
                        # stage the denominator row at partition 0 first
                        den = rpool.tile([1, QW], f32, name="den", tag="den")
                        nc.vector.tensor_copy(den[:], av[A:A + 1, :])
                        nc.vector.reciprocal_approx_fast(out=rec[:],
                                                         in_=den[:])
                    bc = rpool.tile([A, QW], f32, name="bc", tag="bc")
                    if NORM_MODE == "gp":
                        nc.gpsimd.partition_broadcast(bc[:], rec[:])
                    else:
                        rec_r = rpool.tile([1, QW], f32r, name="rec_r",
                                           tag="rec_r")
                        nc.vector.tensor_copy(rec_r[:], rec[:])
                        bcps = psp.tile([A, QW], f32, name="bcps", tag="ps")
                        nc.tensor.matmul(bcps[:], lhsT=ones_r[:],
                                         rhs=rec_r[:], start=True, stop=True)
                        nc.vector.tensor_copy(bc[:], bcps[:])
                    nc.vector.tensor_mul(attn_sb[p][off:off + A, qs],
                                         av[0:A, :], bc[:])

            # ---------------- schedule ----------------
            ob_tiles = [opool.tile([128, D], f32, name=f"ob{i}", tag=f"ob{i}")
                        for i in range(4)]

            for qc in range(QC):
                qk_unit(0, "q", qc)
                qk_unit(0, "k", qc)
            # pair-1 projections fill the PE slack of chunks (0, 1..3)
            for qc in range(QC):
                filler.append(lambda qc=qc: qk_unit(1, "q", qc))
                filler.append(lambda qc=qc: qk_unit(1, "k", qc))

            attention_chunk(0, 0, v_inline=True)
            for qc in range(1, QC):
                attention_chunk(0, qc)
            drain(len(filler))

            for qc in range(QC):
                attention_chunk(1, qc)
                if qc > 0:
                    for mt in range(4 * (qc - 1), 4 * qc):
                        filler.append(lambda mt=mt: fc_unit(mt, 0))
                        filler.append(lambda mt=mt: fc_unit(mt, 1))
            drain(len(filler))
            for mt in range(12, 16):
                fc_unit(mt, 0)
                fc_unit(mt, 1)

    nc.compile()
    return nc


def make_in_maps(x, Wq, bq, Wk, bk, Wv, bv, Wo, bo, att_dt=None):
    import ml_dtypes

    att_dt = att_dt or ATT_DT
    fp8 = att_dt == "fp8"
    f = np.float32
    pdt = f if fp8 else ml_dtypes.bfloat16
    if fp8:
        # channel permutation within a core's 256 columns: channel
        # (h, a) -> partition h*32 + a%32, free half a//32
        perm = np.empty((2, 128), np.int64)
        for h in range(HPG):
            for a in range(A):
                perm[a // 32, h * 32 + (a % 32)] = h * A + a
        cols = np.concatenate([perm[0], perm[1]])
    in_maps = []
    for core in range(N_CORES):
        b, g = divmod(core, GROUPS)
        cs = slice(g * C, (g + 1) * C)
        wq = np.asarray(Wq[:, cs], f)
        wk = np.asarray(Wk[:, cs], f)
        bqc = np.asarray(bq[cs], f)
        bkc = np.asarray(bk[cs], f)
        if fp8:
            wq = wq[:, cols]
            wk = wk[:, cols]
            bqc = bqc[cols]
            bkc = bkc[cols]
        m = {
            "xT": np.ascontiguousarray(np.asarray(x[b]).T.astype(pdt)),
            "wq": np.ascontiguousarray(wq.astype(pdt)),
            "wk": np.ascontiguousarray(wk.astype(pdt)),
            "wv": np.ascontiguousarray(np.asarray(Wv[:, cs], f).astype(pdt)),
            "wo": np.ascontiguousarray(Wo[cs], dtype=f),
            "bqs": np.ascontiguousarray(bqc.reshape(MC, 128).T),
            "bks": np.ascontiguousarray(bkc.reshape(MC, 128).T),
            "bvb": np.ascontiguousarray(np.broadcast_to(bv[cs], (128, C)),
                                        dtype=f),
            "bob": np.ascontiguousarray(
                np.broadcast_to(bo, (128, D)).astype(f) if g == 0
                else np.zeros((128, D), f)),
        }
        in_maps.append(m)
    return in_maps


_nc_cache = {}


def kernel(x, Wq, bq, Wk, bk, Wv, bv, Wo, bo, _trace=False):
    from concourse.bass_utils import run_bass_kernel_spmd

    if "nc" not in _nc_cache:
        _nc_cache["nc"] = build_nc()
    nc = _nc_cache["nc"]
    in_maps = make_in_maps(x, Wq, bq, Wk, bk, Wv, bv, Wo, bo)
    res = run_bass_kernel_spmd(nc, in_maps, core_ids=list(range(N_CORES)),
                               trace=_trace)
    _nc_cache["last_result"] = res
    out = np.empty((B, S, D), np.float32)
    for b in range(B):
        acc = res.results[b * GROUPS]["out"].copy()
        for g in range(1, GROUPS):
            acc += res.results[b * GROUPS + g]["out"]
        out[b] = acc
    return out


# revision 12
# speedup vs baseline: 1.3873x; 1.3873x over previous
"""Multi-head attention (B=2,S=2048,D=1024,H=16,A=64) on 8 trn2 NeuronCores.

Sharding: core = 4*b + g  (b = batch, g = head-group of 4 heads).
Per core: project q/k transposed (feature-on-partition), v natural; per
head-pair S^T tiles -> exp on ACT -> AV accumulate with a ones-column in v
producing the softmax denominator; normalize via fast-reciprocal +
partition-broadcast multiply; fc_out partial (this core's 256 channels of
Wo) interleaved per q-chunk; host sums the 4 partials per batch.

Schedule: attention is ACT(exp)-paced; projections for pair 1, the V
projection, and fc_out chunks are emitted as PE filler inside the
attention chunk loops so the PE never idles waiting on the ACT engine.

ATT_DT: "bf16" = all-bf16 matmuls; "fp8" = scores matmul in fp8e4
DoubleRow (two 32-row halves packed; Wq/Wk columns permuted host-side into
the [32, 2] layout); "fp8av" = fp8 scores + fp8-DR AV (pt/v in e4m3, exp
shifted by e^-1.5 to stay in range).
"""

import numpy as np

B, S, D, H, A = 2, 2048, 1024, 16, 64
GROUPS = 4              # head groups (cores per batch)
HPG = H // GROUPS       # heads per core = 4
C = HPG * A             # channels per core = 256
N_CORES = 8
KD = D // 128           # d-tiles = 8
MC = C // 128           # channel tiles per core = 2
NS = S // 128           # seq tiles = 16
QC = 4                  # q chunks
QW = S // QC            # 512
NG = NS // 2            # S^T groups per chunk (2 k-tiles per group)

ATT_DT = "bf16"         # "bf16" | "fp8" (S^T fp8-DR) | "fp8av" (S^T+AV fp8-DR)
NORM_MODE = "gp"        # "gp": fast recip + gpsimd broadcast;
                        # "pe": fast recip + PE ones-matmul broadcast;
                        # "safe": exact recip + PE broadcast


def build_nc(att_dt=None):
    import concourse.mybir as mybir
    import concourse.tile as tile
    from concourse import bacc

    att_dt = att_dt or ATT_DT
    fp8 = att_dt in ("fp8", "fp8av")
    fp8av = att_dt == "fp8av"
    f32 = mybir.dt.float32
    f32r = mybir.dt.float32r
    bf16 = mybir.dt.bfloat16
    f8 = mybir.dt.float8e4
    AF = mybir.ActivationFunctionType
    DR = mybir.MatmulPerfMode.DoubleRow

    pdt = bf16            # proj input dtype (host-cast)
    avdt = f8 if fp8av else bf16   # AV operand dtype (pt, v values)

    nc = bacc.Bacc(
        "TRN2", target_bir_lowering=False, debug=False,
        enable_asserts=True, num_devices=N_CORES,
    )

    xT_d = nc.dram_tensor("xT", [D, S], pdt, kind="ExternalInput").ap()
    wq_d = nc.dram_tensor("wq", [D, C], pdt, kind="ExternalInput").ap()
    wk_d = nc.dram_tensor("wk", [D, C], pdt, kind="ExternalInput").ap()
    wv_d = nc.dram_tensor("wv", [D, C], pdt, kind="ExternalInput").ap()
    wo_d = nc.dram_tensor("wo", [C, D], bf16, kind="ExternalInput").ap()
    bqs_d = nc.dram_tensor("bqs", [128, MC], f32, kind="ExternalInput").ap()
    bks_d = nc.dram_tensor("bks", [128, MC], f32, kind="ExternalInput").ap()
    bvb_d = nc.dram_tensor("bvb", [128, C], f32, kind="ExternalInput").ap()
    bob_d = nc.dram_tensor("bob", [128, D], f32, kind="ExternalInput").ap()
    out_d = nc.dram_tensor("out", [S, D], f32, kind="ExternalOutput").ap()

    with tile.TileContext(nc) as tc:
        with tc.tile_pool(name="const", bufs=1) as cpool, \
             tc.tile_pool(name="wgt", bufs=1) as wpool, \
             tc.tile_pool(name="qkv", bufs=1) as qpool, \
             tc.tile_pool(name="ptp", bufs=2) as ptpool, \
             tc.tile_pool(name="rcp", bufs=2) as rpool, \
             tc.tile_pool(name="osb", bufs=1) as opool, \
             tc.tile_pool(name="pst", bufs=1, space="PSUM") as stp, \
             tc.tile_pool(name="pav", bufs=1, space="PSUM") as avp, \
             tc.tile_pool(name="psp", bufs=2, space="PSUM") as psp:

            # ---------------- constants + input loads ----------------
            bq_sb = cpool.tile([128, MC], f32, name="bq_sb")
            nc.sync.dma_start(bq_sb[:], bqs_d[:, :])
            bk_sb = cpool.tile([128, MC], f32, name="bk_sb")
            nc.sync.dma_start(bk_sb[:], bks_d[:, :])
            bvb_sb = cpool.tile([128, C], f32, name="bvb_sb")
            nc.sync.dma_start(bvb_sb[:], bvb_d[:, :])
            bob_sb = cpool.tile([128, D], f32, name="bob_sb")
            nc.sync.dma_start(bob_sb[:], bob_d[:, :])

            xT_sb = [wpool.tile([128, S], pdt, name=f"xT{kt}")
                     for kt in range(KD)]
            w_sb = {w: [wpool.tile([128, C], pdt, name=f"w{w}{kt}")
                        for kt in range(KD)]
                    for w in ("q", "k", "v")}
            wds = {"q": wq_d, "k": wk_d, "v": wv_d}
            for kt in range(KD):
                ks = slice(kt * 128, (kt + 1) * 128)
                nc.sync.dma_start(xT_sb[kt][:], xT_d[ks, :])
                for i, w in enumerate(("q", "k", "v")):
                    eng = (nc.gpsimd, nc.scalar, nc.gpsimd)[i]
                    eng.dma_start(w_sb[w][kt][:], wds[w][ks, :])
            wo_sb = [wpool.tile([128, D], bf16, name=f"wo{kt}")
                     for kt in range(MC)]
            for kt in range(MC):
                nc.sync.dma_start(wo_sb[kt][:],
                                  wo_d[kt * 128:(kt + 1) * 128, :])

            if NORM_MODE != "gp":
                ones_f = cpool.tile([1, A], f32, name="ones_f")
                nc.vector.memset(ones_f[:], 1.0)
                ones_r = cpool.tile([1, A], bf16, name="ones_r")
                nc.vector.tensor_copy(ones_r[:], ones_f[:])

            # v padded per head with a ones column: [128, NS, HPG, A+1]
            v_sb = qpool.tile([128, NS, HPG, A + 1], avdt, name="v_sb")
            vones = cpool.tile([128, NS * HPG], f32, name="vones")
            nc.vector.memset(vones[:], 1.0)
            nc.vector.tensor_copy(
                v_sb[:, :, :, A],
                vones[:].rearrange("p (t h) -> p t h", h=HPG))

            if fp8:
                # all 4 heads in one tile: partition = h*32 + a%32,
                # free = (a//32, s)
                qT_f8 = qpool.tile([128, 2, S], f8, name="qT_f8")
                kT_f8 = qpool.tile([128, 2, S], f8, name="kT_f8")
            else:
                qT_sb = [qpool.tile([128, S], bf16, name=f"qT{p}")
                         for p in range(MC)]
                kT_sb = [qpool.tile([128, S], bf16, name=f"kT{p}")
                         for p in range(MC)]
            attn_sb = [qpool.tile([128, S], bf16, name=f"attn{p}")
                       for p in range(MC)]

            # ---------------- work units ----------------
            def qk_unit(mt, wname, qc):
                # one q-chunk of the q or k projection for tile/half mt
                qs = slice(qc * QW, (qc + 1) * QW)
                ps = psp.tile([128, QW], f32, name="ps", tag="ps")
                for kt in range(KD):
                    nc.tensor.matmul(
                        ps[:],
                        lhsT=w_sb[wname][kt][:, mt * 128:(mt + 1) * 128],
                        rhs=xT_sb[kt][:, qs],
                        start=(kt == 0), stop=(kt == KD - 1),
                    )
                bias = (bq_sb if wname == "q" else bk_sb)[:, mt:mt + 1]
                if fp8:
                    dst = qT_f8 if wname == "q" else kT_f8
                    nc.vector.tensor_scalar_add(dst[:, mt, qs], ps[:], bias)
                else:
                    dst = (qT_sb if wname == "q" else kT_sb)[mt]
                    nc.vector.tensor_scalar_add(dst[:, qs], ps[:], bias)

            def v_unit(st):
                # one seq-tile of the v projection (natural layout)
                psv = psp.tile([128, C], f32, name="psv", tag="ps")
                for kt in range(KD):
                    nc.tensor.matmul(
                        psv[:],
                        lhsT=xT_sb[kt][:, st * 128:(st + 1) * 128],
                        rhs=w_sb["v"][kt][:],
                        start=(kt == 0), stop=(kt == KD - 1),
                    )
                nc.vector.tensor_add(
                    v_sb[:, st, :, 0:A],
                    psv[:].rearrange("p (h a) -> p h a", a=A),
                    bvb_sb[:].rearrange("p (h a) -> p h a", a=A),
                )

            def fc_unit(mt, nn):
                # fc_out: s-tile mt (128 rows), d-half nn; contraction over
                # this core's C=256 attn channels
                ps = psp.tile([128, QW], f32, name="psfc", tag="ps")
                for kt in range(MC):
                    nc.tensor.matmul(
                        ps[:],
                        lhsT=attn_sb[kt][:, mt * 128:(mt + 1) * 128],
                        rhs=wo_sb[kt][:, nn * QW:(nn + 1) * QW],
                        start=(kt == 0), stop=(kt == MC - 1),
                    )
                ob = ob_tiles[mt % 4]
                nc.vector.tensor_add(ob[:, nn * QW:(nn + 1) * QW], ps[:],
                                     bob_sb[:, nn * QW:(nn + 1) * QW])
                if nn == 1:
                    nc.sync.dma_start(out_d[mt * 128:(mt + 1) * 128, :],
                                      ob[:])

            # filler queue: zero-arg closures popped between ng groups
            # inside attention chunks to keep the PE busy
            filler = []

            def drain(n):
                for _ in range(n):
                    if filler:
                        filler.pop(0)()

            def attention_chunk(p, qc, v_inline=False):
                heads = (2 * p, 2 * p + 1)
                qs = slice(qc * QW, (qc + 1) * QW)
                avs = [avp.tile([A + 1, QW], f32, name=f"av{hh}",
                                tag=f"av{hh}") for hh in range(2)]
                for ng in range(NG):
                    pts = []
                    for hh in range(2):
                        st = stp.tile([128, 2, QW], f32, name=f"st{hh}",
                                      tag=f"st{hh}")
                        for jj in range(2):
                            kt = 2 * ng + jj
                            if fp8:
                                h = heads[hh]
                                nc.tensor.matmul(
                                    st[:, jj, :],
                                    lhsT=kT_f8[h * 32:(h + 1) * 32, :,
                                               kt * 128:(kt + 1) * 128],
                                    rhs=qT_f8[h * 32:(h + 1) * 32, :, qs],
                                    start=True, stop=True, perf_mode=DR,
                                )
                            else:
                                off = hh * A
                                nc.tensor.matmul(
                                    st[:, jj, :],
                                    lhsT=kT_sb[p][off:off + A,
                                                  kt * 128:(kt + 1) * 128],
                                    rhs=qT_sb[p][off:off + A, qs],
                                    start=True, stop=True,
                                )
                        pt = ptpool.tile([128, 2, QW], avdt, name=f"pt{hh}",
                                         tag=f"pt{hh}")
                        nc.scalar.activation(pt[:], st[:], AF.Exp,
                                             scale=0.125,
                                             bias=-1.5 if fp8av else 0.0)
                        pts.append(pt)
                    if v_inline:
                        v_unit(2 * ng)
                        v_unit(2 * ng + 1)
                    else:
                        drain(1)
                    if fp8av:
                        for hh in range(2):
                            nc.tensor.matmul(
                                avs[hh][:],
                                lhsT=v_sb[:, 2 * ng:2 * ng + 2,
                                          heads[hh], :],
                                rhs=pts[hh][:],
                                start=(ng == 0), stop=(ng == NG - 1),
                                perf_mode=DR,
                            )
                    else:
                        for jj in range(2):
                            kt = 2 * ng + jj
                            for hh in range(2):
                                nc.tensor.matmul(
                                    avs[hh][:],
                                    lhsT=v_sb[:, kt, heads[hh], :],
                                    rhs=pts[hh][:, jj, :],
                                    start=(kt == 0), stop=(kt == NS - 1),
                                )
                # normalize: attn[a, q] = av[a, q] * (1 / av[A, q])
                for hh in range(2):
                    av = avs[hh]
                    off = hh * A
                    rec = rpool.tile([1, QW], f32, name="rec", tag="rec")
                    if NORM_MODE == "safe":
                        nc.vector.reciprocal(rec[:], av[A:A + 1, :])
                    else:
                        # custom-DVE ops drop the input partition offset;
                        # stage the denominator row at partition 0 first
                        den = rpool.tile([1, QW], f32, name="den", tag="den")
                        nc.vector.tensor_copy(den[:], av[A:A + 1, :])
                        nc.vector.reciprocal_approx_fast(out=rec[:],
                                                         in_=den[:])
                    bc = rpool.tile([A, QW], f32, name="bc", tag="bc")
                    if NORM_MODE == "gp":
                        nc.gpsimd.partition_broadcast(bc[:], rec[:])
                    else:
                        rec_r = rpool.tile([1, QW], bf16, name="rec_r",
                                           tag="rec_r")
                        nc.vector.tensor_copy(rec_r[:], rec[:])
                        bcps = psp.tile([A, QW], f32, name="bcps", tag="ps")
                        nc.tensor.matmul(bcps[:], lhsT=ones_r[:],
                                         rhs=rec_r[:], start=True, stop=True)
                        nc.vector.tensor_copy(bc[:], bcps[:])
                    nc.vector.tensor_mul(attn_sb[p][off:off + A, qs],
                                         av[0:A, :], bc[:])

            # ---------------- schedule ----------------
            ob_tiles = [opool.tile([128, D], f32, name=f"ob{i}", tag=f"ob{i}")
                        for i in range(4)]

            for qc in range(QC):
                qk_unit(0, "q", qc)
                qk_unit(0, "k", qc)
            # pair-1 projections fill the PE slack of chunks (0, 1..3)
            for qc in range(QC):
                filler.append(lambda qc=qc: qk_unit(1, "q", qc))
                filler.append(lambda qc=qc: qk_unit(1, "k", qc))

            attention_chunk(0, 0, v_inline=True)
            for qc in range(1, QC):
                attention_chunk(0, qc)
            drain(len(filler))

            for qc in range(QC):
                attention_chunk(1, qc)
                if qc > 0:
                    for mt in range(4 * (qc - 1), 4 * qc):
                        filler.append(lambda mt=mt: fc_unit(mt, 0))
                        filler.append(lambda mt=mt: fc_unit(mt, 1))
            drain(len(filler))
            for mt in range(12, 16):
                fc_unit(mt, 0)
                fc_unit(mt, 1)

    nc.compile()
    return nc


def make_in_maps(x, Wq, bq, Wk, bk, Wv, bv, Wo, bo, att_dt=None):
    import ml_dtypes

    att_dt = att_dt or ATT_DT
    fp8 = att_dt in ("fp8", "fp8av")
    fp8av = att_dt == "fp8av"
    f = np.float32
    pdt = f if fp8 else ml_dtypes.bfloat16
    if fp8:
        # channel permutation within a core's 256 columns: channel
        # (h, a) -> partition h*32 + a%32, free half a//32
        perm = np.empty((2, 128), np.int64)
        for h in range(HPG):
            for a in range(A):
                perm[a // 32, h * 32 + (a % 32)] = h * A + a
        cols = np.concatenate([perm[0], perm[1]])
    in_maps = []
    for core in range(N_CORES):
        b, g = divmod(core, GROUPS)
        cs = slice(g * C, (g + 1) * C)
        wq = np.asarray(Wq[:, cs], f)
        wk = np.asarray(Wk[:, cs], f)
        bqc = np.asarray(bq[cs], f)
        bkc = np.asarray(bk[cs], f)
        if fp8:
            wq = wq[:, cols]
            wk = wk[:, cols]
            bqc = bqc[cols]
            bkc = bkc[cols]
        m = {
            "xT": np.ascontiguousarray(np.asarray(x[b]).T.astype(pdt)),
            "wq": np.ascontiguousarray(wq.astype(pdt)),
            "wk": np.ascontiguousarray(wk.astype(pdt)),
            "wv": np.ascontiguousarray(np.asarray(Wv[:, cs], f).astype(pdt)),
            "wo": np.ascontiguousarray(np.asarray(Wo[cs], f).astype(ml_dtypes.bfloat16)),
            "bqs": np.ascontiguousarray(bqc.reshape(MC, 128).T),
            "bks": np.ascontiguousarray(bkc.reshape(MC, 128).T),
            "bvb": np.ascontiguousarray(np.broadcast_to(bv[cs], (128, C)),
                                        dtype=f),
            "bob": np.ascontiguousarray(
                np.broadcast_to(bo, (128, D)).astype(f) if g == 0
                else np.zeros((128, D), f)),
        }
        in_maps.append(m)
    return in_maps


_nc_cache = {}


def kernel(x, Wq, bq, Wk, bk, Wv, bv, Wo, bo, _trace=False):
    from concourse.bass_utils import run_bass_kernel_spmd

    if "nc" not in _nc_cache:
        _nc_cache["nc"] = build_nc()
    nc = _nc_cache["nc"]
    in_maps = make_in_maps(x, Wq, bq, Wk, bk, Wv, bv, Wo, bo)
    res = run_bass_kernel_spmd(nc, in_maps, core_ids=list(range(N_CORES)),
                               trace=_trace)
    _nc_cache["last_result"] = res
    out = np.empty((B, S, D), np.float32)
    for b in range(B):
        acc = res.results[b * GROUPS]["out"].copy()
        for g in range(1, GROUPS):
            acc += res.results[b * GROUPS + g]["out"]
        out[b] = acc
    return out


# revision 18
# speedup vs baseline: 1.5061x; 1.0857x over previous
"""Multi-head attention (B=2,S=2048,D=1024,H=16,A=64) on 8 trn2 NeuronCores.

Sharding: core = 4*b + g  (b = batch, g = head-group of 4 heads).
Per core: project q/k transposed (feature-on-partition), v natural; per
head-pair S^T tiles -> exp on ACT -> AV accumulate with a ones-column in v
producing the softmax denominator; normalize via fast-reciprocal +
partition-broadcast multiply; fc_out partial (this core's 256 channels of
Wo) interleaved per q-chunk; host sums the 4 partials per batch.

Schedule: attention is ACT(exp)-paced; projections for pair 1, the V
projection, and fc_out chunks are emitted as PE filler inside the
attention chunk loops so the PE never idles waiting on the ACT engine.

ATT_DT: "bf16" = all-bf16 matmuls; "fp8" = scores matmul in fp8e4
DoubleRow (two 32-row halves packed; Wq/Wk columns permuted host-side into
the [32, 2] layout); "fp8av" = fp8 scores + fp8-DR AV (pt/v in e4m3, exp
shifted by e^-1.5 to stay in range).
"""

import numpy as np

B, S, D, H, A = 2, 2048, 1024, 16, 64
GROUPS = 4              # head groups (cores per batch)
HPG = H // GROUPS       # heads per core = 4
C = HPG * A             # channels per core = 256
N_CORES = 8
KD = D // 128           # d-tiles = 8
MC = C // 128           # channel tiles per core = 2
NS = S // 128           # seq tiles = 16
QC = 4                  # q chunks
QW = S // QC            # 512
NG = NS // 2            # S^T groups per chunk (2 k-tiles per group)

ATT_DT = "bf16"         # "bf16" | "fp8" (S^T fp8-DR) | "fp8av" (S^T+AV fp8-DR)
NORM_MODE = "gp"        # "gp": fast recip + gpsimd broadcast;
                        # "pe": fast recip + PE ones-matmul broadcast;
                        # "safe": exact recip + PE broadcast


def build_nc(att_dt=None):
    import concourse.mybir as mybir
    import concourse.tile as tile
    from concourse import bacc

    att_dt = att_dt or ATT_DT
    fp8 = att_dt in ("fp8", "fp8av")
    fp8av = att_dt == "fp8av"
    f32 = mybir.dt.float32
    f32r = mybir.dt.float32r
    bf16 = mybir.dt.bfloat16
    f8 = mybir.dt.float8e4
    AF = mybir.ActivationFunctionType
    DR = mybir.MatmulPerfMode.DoubleRow

    pdt = bf16            # proj input dtype (host-cast)
    avdt = f8 if fp8av else bf16   # AV operand dtype (pt, v values)

    nc = bacc.Bacc(
        "TRN2", target_bir_lowering=False, debug=False,
        enable_asserts=True, num_devices=N_CORES,
    )

    xT_d = nc.dram_tensor("xT", [D, S], pdt, kind="ExternalInput").ap()
    wq_d = nc.dram_tensor("wq", [D, C], pdt, kind="ExternalInput").ap()
    wk_d = nc.dram_tensor("wk", [D, C], pdt, kind="ExternalInput").ap()
    wv_d = nc.dram_tensor("wv", [D, C], pdt, kind="ExternalInput").ap()
    wo_d = nc.dram_tensor("wo", [C, D], bf16, kind="ExternalInput").ap()
    bqs_d = nc.dram_tensor("bqs", [128, MC], f32, kind="ExternalInput").ap()
    bks_d = nc.dram_tensor("bks", [128, MC], f32, kind="ExternalInput").ap()
    bvb_d = nc.dram_tensor("bvb", [128, C], f32, kind="ExternalInput").ap()
    bob_d = nc.dram_tensor("bob", [128, D], f32, kind="ExternalInput").ap()
    out_d = nc.dram_tensor("out", [S, D], f32, kind="ExternalOutput").ap()

    with tile.TileContext(nc) as tc:
        with tc.tile_pool(name="const", bufs=1) as cpool, \
             tc.tile_pool(name="wgt", bufs=1) as wpool, \
             tc.tile_pool(name="qkv", bufs=1) as qpool, \
             tc.tile_pool(name="ptp", bufs=2) as ptpool, \
             tc.tile_pool(name="rcp", bufs=2) as rpool, \
             tc.tile_pool(name="osb", bufs=1) as opool, \
             tc.tile_pool(name="pst", bufs=1, space="PSUM") as stp, \
             tc.tile_pool(name="pav", bufs=1, space="PSUM") as avp, \
             tc.tile_pool(name="psp", bufs=2, space="PSUM") as psp:

            # ---------------- constants + input loads ----------------
            # lead-in critical path: wq (scalar) + xT qc0 + wk (sync) gate
            # the first attention group; wv (gpsimd) gates the v units.
            bq_sb = cpool.tile([128, MC], f32, name="bq_sb")
            nc.sync.dma_start(bq_sb[:], bqs_d[:, :])
            bk_sb = cpool.tile([128, MC], f32, name="bk_sb")
            nc.sync.dma_start(bk_sb[:], bks_d[:, :])

            xT_sb = [wpool.tile([128, S], pdt, name=f"xT{kt}")
                     for kt in range(KD)]
            w_sb = {w: [wpool.tile([128, C], pdt, name=f"w{w}{kt}")
                        for kt in range(KD)]
                    for w in ("q", "k", "v")}
            for kt in range(KD):
                ks = slice(kt * 128, (kt + 1) * 128)
                nc.sync.dma_start(xT_sb[kt][:, 0:QW], xT_d[ks, 0:QW])
                nc.scalar.dma_start(w_sb["q"][kt][:], wq_d[ks, :])
                nc.gpsimd.dma_start(w_sb["v"][kt][:], wv_d[ks, :])
            for kt in range(KD):
                ks = slice(kt * 128, (kt + 1) * 128)
                nc.sync.dma_start(w_sb["k"][kt][:], wk_d[ks, :])
            bvb_sb = cpool.tile([128, C], f32, name="bvb_sb")
            nc.sync.dma_start(bvb_sb[:], bvb_d[:, :])
            for qc in range(1, QC):
                qs = slice(qc * QW, (qc + 1) * QW)
                for kt in range(KD):
                    ks = slice(kt * 128, (kt + 1) * 128)
                    nc.sync.dma_start(xT_sb[kt][:, qs], xT_d[ks, qs])
            wo_sb = [wpool.tile([128, D], bf16, name=f"wo{kt}")
                     for kt in range(MC)]
            for kt in range(MC):
                nc.sync.dma_start(wo_sb[kt][:],
                                  wo_d[kt * 128:(kt + 1) * 128, :])
            bob_sb = cpool.tile([128, D], f32, name="bob_sb")
            nc.sync.dma_start(bob_sb[:], bob_d[:, :])

            if NORM_MODE != "gp":
                ones_f = cpool.tile([1, A], f32, name="ones_f")
                nc.vector.memset(ones_f[:], 1.0)
                ones_r = cpool.tile([1, A], bf16, name="ones_r")
                nc.vector.tensor_copy(ones_r[:], ones_f[:])

            # v padded per head with a ones column: [128, NS, HPG, A+1]
            v_sb = qpool.tile([128, NS, HPG, A + 1], avdt, name="v_sb")
            vones = cpool.tile([128, NS * HPG], f32, name="vones")
            nc.vector.memset(vones[:], 1.0)
            nc.vector.tensor_copy(
                v_sb[:, :, :, A],
                vones[:].rearrange("p (t h) -> p t h", h=HPG))

            if fp8:
                # per-pair tiles: partition = hh*32 + a%32 (hh = head in
                # pair, base 0/32), free = (a//32, s)
                qT_f8 = [qpool.tile([64, 2, S], f8, name=f"qT_f8{p}")
                         for p in range(MC)]
                kT_f8 = [qpool.tile([64, 2, S], f8, name=f"kT_f8{p}")
                         for p in range(MC)]
            else:
                qT_sb = [qpool.tile([128, S], bf16, name=f"qT{p}")
                         for p in range(MC)]
                kT_sb = [qpool.tile([128, S], bf16, name=f"kT{p}")
                         for p in range(MC)]
            attn_sb = [qpool.tile([128, S], bf16, name=f"attn{p}")
                       for p in range(MC)]

            # ---------------- work units ----------------
            def qk_unit(mt, wname, qc):
                # one q-chunk of the q or k projection for tile/half mt
                qs = slice(qc * QW, (qc + 1) * QW)
                ps = psp.tile([128, QW], f32, name="ps", tag="ps")
                for kt in range(KD):
                    nc.tensor.matmul(
                        ps[:],
                        lhsT=w_sb[wname][kt][:, mt * 128:(mt + 1) * 128],
                        rhs=xT_sb[kt][:, qs],
                        start=(kt == 0), stop=(kt == KD - 1),
                    )
                bias = (bq_sb if wname == "q" else bk_sb)[:, mt:mt + 1]
                if fp8:
                    # fp8 W blocks are per PAIR: rows = a_hi*64 + hh*32 + a_lo
                    dst = (qT_f8 if wname == "q" else kT_f8)[mt]
                    for ah in range(2):
                        nc.vector.tensor_scalar_add(
                            dst[:, ah, qs], ps[ah * 64:(ah + 1) * 64, :],
                            bias[ah * 64:(ah + 1) * 64, :])
                else:
                    dst = (qT_sb if wname == "q" else kT_sb)[mt]
                    nc.vector.tensor_scalar_add(dst[:, qs], ps[:], bias)

            def v_unit(st):
                # one seq-tile of the v projection (natural layout)
                psv = psp.tile([128, C], f32, name="psv", tag="ps")
                for kt in range(KD):
                    nc.tensor.matmul(
                        psv[:],
                        lhsT=xT_sb[kt][:, st * 128:(st + 1) * 128],
                        rhs=w_sb["v"][kt][:],
                        start=(kt == 0), stop=(kt == KD - 1),
                    )
                nc.vector.tensor_add(
                    v_sb[:, st, :, 0:A],
                    psv[:].rearrange("p (h a) -> p h a", a=A),
                    bvb_sb[:].rearrange("p (h a) -> p h a", a=A),
                )

            def fc_unit(mt, nn):
                # fc_out: s-tile mt (128 rows), d-half nn; contraction over
                # this core's C=256 attn channels
                ps = psp.tile([128, QW], f32, name="psfc", tag="ps")
                for kt in range(MC):
                    nc.tensor.matmul(
                        ps[:],
                        lhsT=attn_sb[kt][:, mt * 128:(mt + 1) * 128],
                        rhs=wo_sb[kt][:, nn * QW:(nn + 1) * QW],
                        start=(kt == 0), stop=(kt == MC - 1),
                    )
                ob = ob_tiles[mt % 4]
                nc.vector.tensor_add(ob[:, nn * QW:(nn + 1) * QW], ps[:],
                                     bob_sb[:, nn * QW:(nn + 1) * QW])
                if nn == 1:
                    nc.sync.dma_start(out_d[mt * 128:(mt + 1) * 128, :],
                                      ob[:])

            def attention_chunk(p, qc, fills=None):
                # fills: {ng: [closures]} — PE filler units emitted between
                # the exp issue and the AV matmuls of group ng
                fills = fills or {}
                heads = (2 * p, 2 * p + 1)
                qs = slice(qc * QW, (qc + 1) * QW)
                avs = [avp.tile([A + 1, QW], f32, name=f"av{hh}",
                                tag=f"av{hh}") for hh in range(2)]
                for ng in range(NG):
                    pts = []
                    for hh in range(2):
                        st = stp.tile([128, 2, QW], f32, name=f"st{hh}",
                                      tag=f"st{hh}")
                        for jj in range(2):
                            kt = 2 * ng + jj
                            if fp8:
                                hs = slice(hh * 32, (hh + 1) * 32)
                                nc.tensor.matmul(
                                    st[:, jj, :],
                                    lhsT=kT_f8[p][hs, :,
                                                  kt * 128:(kt + 1) * 128],
                                    rhs=qT_f8[p][hs, :, qs],
                                    start=True, stop=True, perf_mode=DR,
                                )
                            else:
                                off = hh * A
                                nc.tensor.matmul(
                                    st[:, jj, :],
                                    lhsT=kT_sb[p][off:off + A,
                                                  kt * 128:(kt + 1) * 128],
                                    rhs=qT_sb[p][off:off + A, qs],
                                    start=True, stop=True,
                                )
                        pt = ptpool.tile([128, 2, QW], avdt, name=f"pt{hh}",
                                         tag=f"pt{hh}")
                        nc.scalar.activation(pt[:], st[:], AF.Exp,
                                             scale=0.125,
                                             bias=-1.5 if fp8av else 0.0)
                        pts.append(pt)
                    for fill in fills.get(ng, ()):
                        fill()
                    if fp8av:
                        for hh in range(2):
                            nc.tensor.matmul(
                                avs[hh][:],
                                lhsT=v_sb[:, 2 * ng:2 * ng + 2,
                                          heads[hh], :],
                                rhs=pts[hh][:],
                                start=(ng == 0), stop=(ng == NG - 1),
                                perf_mode=DR,
                            )
                    else:
                        for jj in range(2):
                            kt = 2 * ng + jj
                            for hh in range(2):
                                nc.tensor.matmul(
                                    avs[hh][:],
                                    lhsT=v_sb[:, kt, heads[hh], :],
                                    rhs=pts[hh][:, jj, :],
                                    start=(kt == 0), stop=(kt == NS - 1),
                                )
                # normalize: attn[a, q] = av[a, q] * (1 / av[A, q])
                for hh in range(2):
                    av = avs[hh]
                    off = hh * A
                    rec = rpool.tile([1, QW], f32, name="rec", tag="rec")
                    if NORM_MODE == "safe":
                        nc.vector.reciprocal(rec[:], av[A:A + 1, :])
                    else:
                        # custom-DVE ops drop the input partition offset;
                        # stage the denominator row at partition 0 first
                        den = rpool.tile([1, QW], f32, name="den", tag="den")
                        nc.vector.tensor_copy(den[:], av[A:A + 1, :])
                        nc.vector.reciprocal_approx_fast(out=rec[:],
                                                         in_=den[:])
                    bc = rpool.tile([A, QW], f32, name="bc", tag="bc")
                    if NORM_MODE == "gp":
                        nc.gpsimd.partition_broadcast(bc[:], rec[:])
                    else:
                        rec_r = rpool.tile([1, QW], bf16, name="rec_r",
                                           tag="rec_r")
                        nc.vector.tensor_copy(rec_r[:], rec[:])
                        bcps = psp.tile([A, QW], f32, name="bcps", tag="ps")
                        nc.tensor.matmul(bcps[:], lhsT=ones_r[:],
                                         rhs=rec_r[:], start=True, stop=True)
                        nc.vector.tensor_copy(bc[:], bcps[:])
                    nc.vector.tensor_mul(attn_sb[p][off:off + A, qs],
                                         av[0:A, :], bc[:])

            # ---------------- schedule ----------------
            ob_tiles = [opool.tile([128, D], f32, name=f"ob{i}", tag=f"ob{i}")
                        for i in range(4)]

            def qk(mt, w, qc):
                return lambda: qk_unit(mt, w, qc)

            def vu(st):
                return lambda: v_unit(st)

            def fc(mt, nn):
                return lambda: fc_unit(mt, nn)

            # minimal lead-in: only the (q, k) qc0 projections of pair 0,
            # then attention starts; everything else is placed filler.
            qk_unit(0, "q", 0)
            qk_unit(0, "k", 0)
            # chunk (0,0): v units just-in-time for AV; k0(qc) due before
            # S^T group 2*qc; q0 tail spread late
            attention_chunk(0, 0, fills={
                0: [vu(0), vu(1)],
                1: [vu(2), vu(3), qk(0, "k", 1)],
                2: [vu(4), vu(5)],
                3: [vu(6), vu(7), qk(0, "k", 2)],
                4: [vu(8), vu(9)],
                5: [vu(10), vu(11), qk(0, "k", 3)],
                6: [vu(12), vu(13), qk(0, "q", 1)],
                7: [vu(14), vu(15)],
            })
            attention_chunk(0, 1, fills={
                0: [qk(0, "q", 2)],
                1: [qk(1, "k", 0)],
                3: [qk(1, "k", 1)],
                4: [qk(0, "q", 3)],
                5: [qk(1, "k", 2)],
                7: [qk(1, "k", 3)],
            })
            attention_chunk(0, 2, fills={
                0: [qk(1, "q", 0)],
                3: [qk(1, "q", 1)],
            })
            attention_chunk(0, 3, fills={
                1: [qk(1, "q", 2)],
                4: [qk(1, "q", 3)],
            })
            attention_chunk(1, 0)
            for qc in range(1, QC):
                # fc over the previous q chunk: 8 units across 8 groups
                attention_chunk(1, qc, fills={
                    ng: [fc(4 * (qc - 1) + ng // 2, ng % 2)]
                    for ng in range(NG)
                })
            for mt in range(12, 16):
                fc_unit(mt, 0)
                fc_unit(mt, 1)

    nc.compile()
    return nc


def make_in_maps(x, Wq, bq, Wk, bk, Wv, bv, Wo, bo, att_dt=None):
    import ml_dtypes

    att_dt = att_dt or ATT_DT
    fp8 = att_dt in ("fp8", "fp8av")
    fp8av = att_dt == "fp8av"
    f = np.float32
    pdt = f if fp8 else ml_dtypes.bfloat16
    if fp8:
        # column blocks per PAIR: block p col a_hi*64 + hh*32 + a_lo holds
        # channel (2p+hh)*64 + a_hi*32 + a_lo
        cols = np.empty(C, np.int64)
        for p in range(MC):
            for ah in range(2):
                for hh in range(2):
                    for al in range(32):
                        cols[p * 128 + ah * 64 + hh * 32 + al] = \
                            (2 * p + hh) * A + ah * 32 + al
    in_maps = []
    for core in range(N_CORES):
        b, g = divmod(core, GROUPS)
        cs = slice(g * C, (g + 1) * C)
        wq = np.asarray(Wq[:, cs], f)
        wk = np.asarray(Wk[:, cs], f)
        bqc = np.asarray(bq[cs], f)
        bkc = np.asarray(bk[cs], f)
        if fp8:
            wq = wq[:, cols]
            wk = wk[:, cols]
            bqc = bqc[cols]
            bkc = bkc[cols]
        m = {
            "xT": np.ascontiguousarray(np.asarray(x[b]).T.astype(pdt)),
            "wq": np.ascontiguousarray(wq.astype(pdt)),
            "wk": np.ascontiguousarray(wk.astype(pdt)),
            "wv": np.ascontiguousarray(np.asarray(Wv[:, cs], f).astype(pdt)),
            "wo": np.ascontiguousarray(np.asarray(Wo[cs], f).astype(ml_dtypes.bfloat16)),
            "bqs": np.ascontiguousarray(bqc.reshape(MC, 128).T),
            "bks": np.ascontiguousarray(bkc.reshape(MC, 128).T),
            "bvb": np.ascontiguousarray(np.broadcast_to(bv[cs], (128, C)),
                                        dtype=f),
            "bob": np.ascontiguousarray(
                np.broadcast_to(bo, (128, D)).astype(f) if g == 0
                else np.zeros((128, D), f)),
        }
        in_maps.append(m)
    return in_maps


_nc_cache = {}


def kernel(x, Wq, bq, Wk, bk, Wv, bv, Wo, bo, _trace=False):
    from concourse.bass_utils import run_bass_kernel_spmd

    if "nc" not in _nc_cache:
        _nc_cache["nc"] = build_nc()
    nc = _nc_cache["nc"]
    in_maps = make_in_maps(x, Wq, bq, Wk, bk, Wv, bv, Wo, bo)
    res = run_bass_kernel_spmd(nc, in_maps, core_ids=list(range(N_CORES)),
                               trace=_trace)
    _nc_cache["last_result"] = res
    out = np.empty((B, S, D), np.float32)
    for b in range(B):
        acc = res.results[b * GROUPS]["out"].copy()
        for g in range(1, GROUPS):
            acc += res.results[b * GROUPS + g]["out"]
        out[b] = acc
    return out
